# revision 5
# baseline (speedup 1.0000x reference)
"""COMA loss kernel v2 — orientation B (N on partitions) with PE reductions.

Layout: per core, partition p = parity*64 + n (t = 2j + parity), free =
(j, ba) with ba = local (b, a) row, BA = 128. The six per-(t, ba) sums
(sum_e, e.q, e.lg, oh.q, oh.tq, oh.lg) are computed by streaming the six
product slabs through the PE as 128-column lhsT blocks against a
stationary ones[128, 2] rhs: each matmul yields [128 = ba, 2 = parity]
partial sums directly in the final [BA, t, quantity] layout in PSUM.

Element-wise products run bf16 on DVE (2x mode) and Pool; exp on ACT.
Stage 2 (per-(t,ba) scalar math + lambda scan) is identical to v1,
operating on [BA, T] f32 tiles.
"""

import sys

for _p in ("/opt/trn_rl_repo",):
    if _p not in sys.path:
        sys.path.insert(0, _p)

import numpy as np

import concourse.bass as bass
import concourse.bacc as bacc
import concourse.mybir as mybir
from concourse.bass_utils import run_bass_kernel_spmd
from concourse.tile import TileContext

T, B, A, N = 256, 128, 8, 64
M = 8                 # cores
BL = B // M           # local batch
BA = BL * A           # 128 rows
TH = T // 2           # 128 t-pairs
TCH = 8               # j's (t-pairs) per chunk
NCH = TH // TCH       # 16 chunks
GAMMA, LAMBDA = 0.99, 0.95

F32 = mybir.dt.float32
BF16 = mybir.dt.bfloat16
NPBF16 = mybir.dt.np(BF16)


def build_program(ablate: str = "") -> bass.Bass:
    """ablate: comma-set of {muls, mm, stage2} to drop (sim ablations only)."""
    abl = set(ablate.split(",")) if ablate else set()
    nc = bacc.Bacc("TRN2", target_bir_lowering=False, debug=False)

    # big inputs, orientation B: [p = parity*64+n, j, ba]
    lg_d = nc.dram_tensor("lg", [128, TH, BA], BF16, kind="ExternalInput")
    qv_d = nc.dram_tensor("qv", [128, TH, BA], BF16, kind="ExternalInput")
    tq_d = nc.dram_tensor("tq", [128, TH, BA], BF16, kind="ExternalInput")
    oh_d = nc.dram_tensor("oh", [128, TH, BA], BF16, kind="ExternalInput")
    # small inputs, orientation A: [ba, t]
    wgt = nc.dram_tensor("wgt", [BA, T], F32, kind="ExternalInput")
    rwd = nc.dram_tensor("rwd", [BA, T], F32, kind="ExternalInput")
    ones2_d = nc.dram_tensor("ones2", [128, 2], BF16, kind="ExternalInput")
    out = nc.dram_tensor("out", [BA, 3], F32, kind="ExternalOutput")

    AX = mybir.AxisListType.X
    OP = mybir.AluOpType

    with TileContext(nc) as tc:
        with (
            tc.tile_pool(name="inp", bufs=2) as inp,
            tc.tile_pool(name="prd", bufs=2) as prd,
            tc.tile_pool(name="per", bufs=1) as per,
            tc.tile_pool(name="ps", bufs=2, space="PSUM") as pspool,
        ):
            # parity selector: ones2[c, par] = 1 iff c//64 == par
            ones2 = per.tile([128, 2], BF16)
            nc.sync.dma_start(out=ones2[:], in_=ones2_d[:])

            w_t = per.tile([BA, T], F32)
            nc.sync.dma_start(out=w_t[:], in_=wgt[:])
            r_t = per.tile([BA, T], F32)
            nc.sync.dma_start(out=r_t[:], in_=rwd[:])

            # fields[ba, j, 2q+par] = quantity q at t=2j+par, f32
            # q: 0=sum_e 1=dot_eq 2=dot_el 3=q_tk 4=tq_tk 5=l_tk
            fields = per.tile([BA, TH, 12], F32)

            for c in range(NCH):
                sl = slice(c * TCH, (c + 1) * TCH)

                lg = inp.tile([128, TCH, BA], BF16, tag="lg")
                qv = inp.tile([128, TCH, BA], BF16, tag="qv")
                tq = inp.tile([128, TCH, BA], BF16, tag="tq")
                oh = inp.tile([128, TCH, BA], BF16, tag="oh")
                nc.sync.dma_start(out=lg[:], in_=lg_d[:, sl])
                nc.sync.dma_start(out=qv[:], in_=qv_d[:, sl])
                nc.sync.dma_start(out=tq[:], in_=tq_d[:, sl])
                nc.sync.dma_start(out=oh[:], in_=oh_d[:, sl])

                # P[:, q] = the six streams: e, e*q, e*lg, oh*q, oh*tq, oh*lg
                P = prd.tile([128, 6, TCH, BA], BF16, tag="P")
                nc.scalar.activation(
                    out=P[:, 0], in_=lg[:], func=mybir.ActivationFunctionType.Exp
                )
                if "muls" not in abl:
                    # DVE (bf16 2x): e*lg, oh*lg, oh*tq
                    nc.vector.tensor_mul(P[:, 2], P[:, 0], lg[:])
                    nc.vector.tensor_mul(P[:, 5], oh[:], lg[:])
                    nc.vector.tensor_mul(P[:, 4], oh[:], tq[:])
                    # Pool: e*q, oh*q
                    nc.gpsimd.tensor_mul(P[:, 1], P[:, 0], qv[:])
                    nc.gpsimd.tensor_mul(P[:, 3], oh[:], qv[:])

                if "mm" not in abl:
                    psum = pspool.tile([128, TCH, 12], F32, tag="acc")
                    for j in range(TCH):
                        for q in range(6):
                            nc.tensor.matmul(
                                psum[:, j, 2 * q : 2 * q + 2],
                                P[:, q, j, :],
                                ones2[:],
                                start=True,
                                stop=True,
                            )
                    nc.scalar.copy(out=fields[:, sl], in_=psum[:])
                else:
                    nc.scalar.copy(
                        out=fields[:, sl],
                        in_=P[:, 0:1, :, 0:12].rearrange("p q a b -> p (q a) b"),
                    )
            if "stage2" in abl:
                partials = per.tile([BA, 3], F32)
                nc.vector.tensor_copy(partials[:], fields[:, 0, 0:3])
                nc.sync.dma_start(out=out[:], in_=partials[:])
                return nc

            # ---- stage 2: per-(t,ba) scalar math on [BA, T] f32 ----------
            # contiguous t-ordered copies of the six quantities
            def tcopy(dst, q):
                # fields[:, :, 2q:2q+2] -> dst [BA, T] in t-order (t = 2j+par)
                nc.vector.tensor_copy(
                    dst[:].rearrange("p (a b) -> p a b", a=TH, b=2),
                    fields[:, :, 2 * q : 2 * q + 2],
                )

            sum_e = per.tile([BA, T], F32)
            tcopy(sum_e, 0)
            dot_eq = per.tile([BA, T], F32)
            tcopy(dot_eq, 1)
            dot_el = per.tile([BA, T], F32)
            tcopy(dot_el, 2)
            q_tk = per.tile([BA, T], F32)
            tcopy(q_tk, 3)
            tq_tk = per.tile([BA, T], F32)
            tcopy(tq_tk, 4)
            l_tk = per.tile([BA, T], F32)
            tcopy(l_tk, 5)

            z = per.tile([BA, T], F32)  # logsumexp
            nc.scalar.activation(
                out=z[:], in_=sum_e[:], func=mybir.ActivationFunctionType.Ln
            )
            rs = per.tile([BA, T], F32)  # 1/sum_e
            nc.vector.reciprocal(rs[:], sum_e[:])

            logp = per.tile([BA, T], F32)
            nc.vector.tensor_tensor(out=logp[:], in0=l_tk[:], in1=z[:], op=OP.subtract)
            bl = per.tile([BA, T], F32)  # baseline = dot_eq / sum_e
            nc.vector.tensor_mul(bl[:], dot_eq[:], rs[:])
            adv = per.tile([BA, T], F32)
            nc.vector.tensor_tensor(out=adv[:], in0=q_tk[:], in1=bl[:], op=OP.subtract)
            ent = per.tile([BA, T], F32)  # entropy = z - dot_el / sum_e
            nc.vector.tensor_mul(ent[:], dot_el[:], rs[:])
            nc.vector.tensor_tensor(out=ent[:], in0=z[:], in1=ent[:], op=OP.subtract)

            pol = per.tile([BA, T], F32)  # logp * adv * w
            nc.vector.tensor_mul(pol[:], logp[:], adv[:])
            nc.vector.tensor_mul(pol[:], pol[:], w_t[:])
            entw = per.tile([BA, T], F32)
            nc.vector.tensor_mul(entw[:], ent[:], w_t[:])

            # lambda returns (reverse-time scan via negative-step views)
            d = per.tile([BA, T - 1], F32)
            nc.vector.tensor_scalar_mul(d[:], tq_tk[:, 1:T], GAMMA * (1.0 - LAMBDA))
            nc.vector.tensor_add(d[:], d[:], r_t[:, 0 : T - 1])
            gl = per.tile([BA, 1], F32)
            nc.vector.memset(gl[:], GAMMA * LAMBDA)
            ret = per.tile([BA, T - 1], F32)
            nc.vector.tensor_tensor_scan(
                out=ret[:, ::-1],
                data0=gl[:].to_broadcast([BA, T - 1]),
                data1=d[:, ::-1],
                initial=tq_tk[:, T - 1 : T],
                op0=OP.mult,
                op1=OP.add,
            )

            qd = per.tile([BA, T - 1], F32)
            nc.vector.tensor_tensor(
                out=qd[:], in0=ret[:], in1=q_tk[:, 0 : T - 1], op=OP.subtract
            )
            nc.vector.tensor_mul(qd[:], qd[:], qd[:])
            nc.vector.tensor_mul(qd[:], qd[:], w_t[:, 0 : T - 1])

            partials = per.tile([BA, 3], F32)
            nc.vector.reduce_sum(out=partials[:, 0:1], in_=pol[:], axis=AX)
            nc.vector.reduce_sum(out=partials[:, 1:2], in_=qd[:], axis=AX)
            nc.vector.reduce_sum(out=partials[:, 2:3], in_=entw[:], axis=AX)
            nc.sync.dma_start(out=out[:], in_=partials[:])

    return nc


def _orientB(x):
    """[T, BA, N] -> [parity*64+n, T//2, BA] bf16 contiguous."""
    y = x.reshape(T // 2, 2, BA, N).transpose(1, 3, 0, 2).reshape(128, TH, BA)
    return np.ascontiguousarray(y.astype(NPBF16))


def make_in_maps(logit, action, q_value, target_q_value, reward, weight):
    logit = np.asarray(logit, np.float32)
    q_value = np.asarray(q_value, np.float32)
    target_q_value = np.asarray(target_q_value, np.float32)
    action = np.asarray(action)
    reward = np.asarray(reward, np.float32)
    weight = np.asarray(weight, np.float32)

    onehot = (action[..., None] == np.arange(N)).astype(np.float32)  # [T,B,A,N]

    ones2 = np.zeros((128, 2), np.float32)
    ones2[:64, 0] = 1.0
    ones2[64:, 1] = 1.0
    ones2 = ones2.astype(NPBF16)

    in_maps = []
    for r in range(M):
        bs, be = r * BL, (r + 1) * BL
        in_maps.append(
            {
                "ones2": ones2,
                "lg": _orientB(logit[:, bs:be].reshape(T, BA, N)),
                "qv": _orientB(q_value[:, bs:be].reshape(T, BA, N)),
                "tq": _orientB(target_q_value[:, bs:be].reshape(T, BA, N)),
                "oh": _orientB(onehot[:, bs:be].reshape(T, BA, N)),
                "wgt": np.ascontiguousarray(weight[:, bs:be].reshape(T, BA).T),
                "rwd": np.ascontiguousarray(
                    np.repeat(reward[:, bs:be], A, axis=1).T
                ),
            }
        )
    return in_maps


def combine_partials(partials_per_core):
    s = np.stack(partials_per_core).astype(np.float64).sum(axis=(0, 1))
    policy_loss = np.float32(-s[0] / (T * B * A))
    q_value_loss = np.float32(s[1] / ((T - 1) * B * A))
    entropy_loss = np.float32(s[2] / (T * B * A))
    return policy_loss, q_value_loss, entropy_loss


_program_cache = {}


def _get_program() -> bass.Bass:
    if "nc" not in _program_cache:
        nc = build_program()
        nc.finalize()
        _program_cache["nc"] = nc
    return _program_cache["nc"]


def kernel(logit, action, q_value, target_q_value, reward, weight):
    nc = _get_program()
    in_maps = make_in_maps(logit, action, q_value, target_q_value, reward, weight)
    res = run_bass_kernel_spmd(nc, in_maps, list(range(M))).results
    return combine_partials([np.asarray(res[i]["out"]) for i in range(M)])


# revision 16
# speedup vs baseline: 1.1862x; 1.1862x over previous
"""COMA loss kernel v2 — orientation B (N on partitions) with PE reductions.

Layout: per core, partition p = parity*64 + n (t = 2j + parity), free =
(j, ba) with ba = local (b, a) row, BA = 128. The six per-(t, ba) sums
(sum_e, e.q, e.lg, oh.q, oh.tq, oh.lg) are computed by streaming the six
product slabs through the PE as 128-column lhsT blocks against a
stationary ones[128, 2] rhs: each matmul yields [128 = ba, 2 = parity]
partial sums directly in the final [BA, t, quantity] layout in PSUM.

Element-wise products run bf16 on DVE (2x mode) and Pool; exp on ACT.
Stage 2 (per-(t,ba) scalar math + lambda scan) is identical to v1,
operating on [BA, T] f32 tiles.
"""

import sys

for _p in ("/opt/trn_rl_repo",):
    if _p not in sys.path:
        sys.path.insert(0, _p)

import numpy as np

import concourse.bass as bass
import concourse.bacc as bacc
import concourse.mybir as mybir
from concourse.bass_utils import run_bass_kernel_spmd
from concourse.tile import TileContext

T, B, A, N = 256, 128, 8, 64
M = 8                 # cores
BL = B // M           # local batch
BA = BL * A           # 128 rows
TH = T // 2           # 128 t-pairs
TCH = 16              # j's (t-pairs) per chunk
NCH = TH // TCH       # 16 chunks
GAMMA, LAMBDA = 0.99, 0.95

F32 = mybir.dt.float32
BF16 = mybir.dt.bfloat16
F8 = mybir.dt.float8e4
NPBF16 = mybir.dt.np(BF16)
NPF8 = mybir.dt.np(F8)


def build_program(ablate: str = "") -> bass.Bass:
    """ablate: comma-set of {muls, mm, stage2} to drop (sim ablations only)."""
    abl = set(ablate.split(",")) if ablate else set()
    nc = bacc.Bacc("TRN2", target_bir_lowering=False, debug=False)

    # big inputs, orientation B: [p = parity*64+n, j, ba]
    lg_d = nc.dram_tensor("lg", [128, TH, BA], BF16, kind="ExternalInput")
    qv_d = nc.dram_tensor("qv", [128, TH, BA], BF16, kind="ExternalInput")
    tq_d = nc.dram_tensor("tq", [128, TH, BA], F8, kind="ExternalInput")
    oh_d = nc.dram_tensor("oh", [128, TH, BA], F8, kind="ExternalInput")
    # small inputs, orientation A: [ba, t]
    wgt = nc.dram_tensor("wgt", [BA, T], F32, kind="ExternalInput")
    rwd = nc.dram_tensor("rwd", [BA, T], F32, kind="ExternalInput")
    ones2_d = nc.dram_tensor("ones2", [128, 2], BF16, kind="ExternalInput")
    out = nc.dram_tensor("out", [BA, 3], F32, kind="ExternalOutput")

    AX = mybir.AxisListType.X
    OP = mybir.AluOpType

    with TileContext(nc) as tc:
        with (
            tc.tile_pool(name="inp", bufs=2) as inp,
            tc.tile_pool(name="prd", bufs=2) as prd,
            tc.tile_pool(name="per", bufs=1) as per,
            tc.tile_pool(name="ps", bufs=4, space="PSUM") as pspool,
        ):
            # parity selector: ones2[c, par] = 1 iff c//64 == par
            ones2 = per.tile([128, 2], BF16)
            nc.sync.dma_start(out=ones2[:], in_=ones2_d[:])

            w_t = per.tile([BA, T], F32)
            nc.sync.dma_start(out=w_t[:], in_=wgt[:])
            r_t = per.tile([BA, T], F32)
            nc.sync.dma_start(out=r_t[:], in_=rwd[:])

            # fields[ba, j, 2q+par] = quantity q at t=2j+par, f32
            # q: 0=sum_e 1=dot_eq 2=dot_el 3=q_tk 4=tq_tk 5=l_tk
            fields = per.tile([BA, TH, 12], F32)

            for c in range(NCH):
                sl = slice(c * TCH, (c + 1) * TCH)

                lg = inp.tile([128, TCH, BA], BF16, tag="lg")
                qv = inp.tile([128, TCH, BA], BF16, tag="qv")
                tq = inp.tile([128, TCH, BA], F8, tag="tq")
                oh = inp.tile([128, TCH, BA], F8, tag="oh")
                nc.sync.dma_start(out=lg[:], in_=lg_d[:, sl])
                nc.sync.dma_start(out=qv[:], in_=qv_d[:, sl])
                nc.sync.dma_start(out=tq[:], in_=tq_d[:, sl])
                nc.sync.dma_start(out=oh[:], in_=oh_d[:, sl])

                # P[:, q] = the six streams: e, e*q, e*lg, oh*q, oh*tq, oh*lg
                P = prd.tile([128, 6, TCH, BA], BF16, tag="P")
                nc.scalar.activation(
                    out=P[:, 0], in_=lg[:], func=mybir.ActivationFunctionType.Exp
                )
                if "muls" not in abl:
                    # DVE: e*lg, e*q (bf16 2x), oh*q (fp8 mixed, 1x)
                    nc.vector.tensor_mul(P[:, 2], P[:, 0], lg[:])
                    nc.vector.tensor_mul(P[:, 1], P[:, 0], qv[:])
                    nc.vector.tensor_mul(P[:, 3], oh[:], qv[:])
                    # Pool: oh*tq, oh*lg
                    nc.gpsimd.tensor_mul(P[:, 4], oh[:], tq[:])
                    nc.gpsimd.tensor_mul(P[:, 5], oh[:], lg[:])

                if "mm" not in abl:
                    psum = pspool.tile([128, TCH, 12], F32, tag="acc")
                    for j in range(TCH):
                        for q in range(6):
                            nc.tensor.matmul(
                                psum[:, j, 2 * q : 2 * q + 2],
                                P[:, q, j, :],
                                ones2[:],
                                start=True,
                                stop=True,
                            )
                    nc.scalar.copy(out=fields[:, sl], in_=psum[:])
                else:
                    nc.scalar.copy(
                        out=fields[:, sl],
                        in_=P[:, 0:1, :, 0:12].rearrange("p q a b -> p (q a) b"),
                    )
            if "stage2" in abl:
                partials = per.tile([BA, 3], F32)
                nc.vector.tensor_copy(partials[:], fields[:, 0, 0:3])
                nc.sync.dma_start(out=out[:], in_=partials[:])
                return nc

            # ---- stage 2: per-(t,ba) scalar math on [BA, T] f32 ----------
            # contiguous t-ordered copies of the six quantities
            # one batched copy: quant[ba, q, t] (t = 2j+par) from fields
            quant = per.tile([BA, 6, T], F32)
            nc.vector.tensor_copy(
                quant[:].rearrange("p q (a b) -> p q a b", a=TH, b=2),
                fields[:]
                .rearrange("p a (q b) -> p q a b", q=6, b=2),
            )
            sum_e = quant[:, 0]
            dot_eq = quant[:, 1]
            dot_el = quant[:, 2]
            q_tk = quant[:, 3]
            tq_tk = quant[:, 4]
            l_tk = quant[:, 5]

            z = per.tile([BA, T], F32)  # logsumexp
            nc.scalar.activation(
                out=z[:], in_=sum_e[:], func=mybir.ActivationFunctionType.Ln
            )
            rs = per.tile([BA, T], F32)  # 1/sum_e
            nc.vector.reciprocal(rs[:], sum_e[:])

            logp = per.tile([BA, T], F32)
            nc.vector.tensor_tensor(out=logp[:], in0=l_tk[:], in1=z[:], op=OP.subtract)
            bl = per.tile([BA, T], F32)  # baseline = dot_eq / sum_e
            nc.vector.tensor_mul(bl[:], dot_eq[:], rs[:])
            adv = per.tile([BA, T], F32)
            nc.vector.tensor_tensor(out=adv[:], in0=q_tk[:], in1=bl[:], op=OP.subtract)
            ent = per.tile([BA, T], F32)  # entropy = z - dot_el / sum_e
            nc.vector.tensor_mul(ent[:], dot_el[:], rs[:])
            nc.vector.tensor_tensor(out=ent[:], in0=z[:], in1=ent[:], op=OP.subtract)

            pol = per.tile([BA, T], F32)  # logp * adv * w
            nc.vector.tensor_mul(pol[:], logp[:], adv[:])
            nc.vector.tensor_mul(pol[:], pol[:], w_t[:])
            entw = per.tile([BA, T], F32)
            nc.gpsimd.tensor_mul(entw[:], ent[:], w_t[:])

            # lambda returns (reverse-time scan via negative-step views)
            d = per.tile([BA, T - 1], F32)
            nc.gpsimd.tensor_scalar_mul(d[:], tq_tk[:, 1:T], GAMMA * (1.0 - LAMBDA))
            nc.gpsimd.tensor_add(d[:], d[:], r_t[:, 0 : T - 1])
            gl = per.tile([BA, 1], F32)
            nc.vector.memset(gl[:], GAMMA * LAMBDA)
            ret = per.tile([BA, T - 1], F32)
            nc.vector.tensor_tensor_scan(
                out=ret[:, ::-1],
                data0=gl[:].to_broadcast([BA, T - 1]),
                data1=d[:, ::-1],
                initial=tq_tk[:, T - 1 : T],
                op0=OP.mult,
                op1=OP.add,
            )

            qd = per.tile([BA, T - 1], F32)
            nc.vector.tensor_tensor(
                out=qd[:], in0=ret[:], in1=q_tk[:, 0 : T - 1], op=OP.subtract
            )
            nc.scalar.square(qd[:], qd[:])
            nc.vector.tensor_mul(qd[:], qd[:], w_t[:, 0 : T - 1])

            partials = per.tile([BA, 3], F32)
            nc.vector.reduce_sum(out=partials[:, 0:1], in_=pol[:], axis=AX)
            nc.vector.reduce_sum(out=partials[:, 1:2], in_=qd[:], axis=AX)
            nc.vector.reduce_sum(out=partials[:, 2:3], in_=entw[:], axis=AX)
            nc.sync.dma_start(out=out[:], in_=partials[:])

    return nc


def _orientB(x, dtype=None):
    """[T, BA, N] -> [parity*64+n, T//2, BA] contiguous."""
    y = x.reshape(T // 2, 2, BA, N).transpose(1, 3, 0, 2).reshape(128, TH, BA)
    return np.ascontiguousarray(y.astype(dtype if dtype is not None else NPBF16))


def make_in_maps(logit, action, q_value, target_q_value, reward, weight):
    logit = np.asarray(logit, np.float32)
    q_value = np.asarray(q_value, np.float32)
    target_q_value = np.asarray(target_q_value, np.float32)
    action = np.asarray(action)
    reward = np.asarray(reward, np.float32)
    weight = np.asarray(weight, np.float32)

    onehot = (action[..., None] == np.arange(N)).astype(np.float32)  # [T,B,A,N]

    ones2 = np.zeros((128, 2), np.float32)
    ones2[:64, 0] = 1.0
    ones2[64:, 1] = 1.0
    ones2 = ones2.astype(NPBF16)

    in_maps = []
    for r in range(M):
        bs, be = r * BL, (r + 1) * BL
        in_maps.append(
            {
                "ones2": ones2,
                "lg": _orientB(logit[:, bs:be].reshape(T, BA, N)),
                "qv": _orientB(q_value[:, bs:be].reshape(T, BA, N)),
                "tq": _orientB(target_q_value[:, bs:be].reshape(T, BA, N), NPF8),
                "oh": _orientB(onehot[:, bs:be].reshape(T, BA, N), NPF8),
                "wgt": np.ascontiguousarray(weight[:, bs:be].reshape(T, BA).T),
                "rwd": np.ascontiguousarray(
                    np.repeat(reward[:, bs:be], A, axis=1).T
                ),
            }
        )
    return in_maps


def combine_partials(partials_per_core):
    s = np.stack(partials_per_core).astype(np.float64).sum(axis=(0, 1))
    policy_loss = np.float32(-s[0] / (T * B * A))
    q_value_loss = np.float32(s[1] / ((T - 1) * B * A))
    entropy_loss = np.float32(s[2] / (T * B * A))
    return policy_loss, q_value_loss, entropy_loss


_program_cache = {}


def _get_program() -> bass.Bass:
    if "nc" not in _program_cache:
        nc = build_program()
        nc.finalize()
        _program_cache["nc"] = nc
    return _program_cache["nc"]


def kernel(logit, action, q_value, target_q_value, reward, weight):
    nc = _get_program()
    in_maps = make_in_maps(logit, action, q_value, target_q_value, reward, weight)
    res = run_bass_kernel_spmd(nc, in_maps, list(range(M))).results
    return combine_partials([np.asarray(res[i]["out"]) for i in range(M)])


# revision 35
# speedup vs baseline: 1.1957x; 1.0080x over previous
"""COMA loss kernel v2 — orientation B (N on partitions) with PE reductions.

Layout: per core, partition p = parity*64 + n (t = 2j + parity), free =
(j, ba) with ba = local (b, a) row, BA = 128. The six per-(t, ba) sums
(sum_e, e.q, e.lg, oh.q, oh.tq, oh.lg) are computed by streaming the six
product slabs through the PE as 128-column lhsT blocks against a
stationary ones[128, 2] rhs: each matmul yields [128 = ba, 2 = parity]
partial sums directly in the final [BA, t, quantity] layout in PSUM.

Element-wise products run bf16 on DVE (2x mode) and Pool; exp on ACT.
Stage 2 (per-(t,ba) scalar math + lambda scan) is identical to v1,
operating on [BA, T] f32 tiles.
"""

import sys

for _p in ("/opt/trn_rl_repo",):
    if _p not in sys.path:
        sys.path.insert(0, _p)

import numpy as np

import concourse.bass as bass
import concourse.bacc as bacc
import concourse.mybir as mybir
from concourse.bass_utils import run_bass_kernel_spmd
from concourse.tile import TileContext

T, B, A, N = 256, 128, 8, 64
M = 8                 # cores
BL = B // M           # local batch
BA = BL * A           # 128 rows
TH = T // 2           # 128 t-pairs
TCH = 16              # j's (t-pairs) per chunk
NCH = TH // TCH       # 16 chunks
GAMMA, LAMBDA = 0.99, 0.95

F32 = mybir.dt.float32
BF16 = mybir.dt.bfloat16
F8 = mybir.dt.float8e4
U8 = mybir.dt.uint8
NPBF16 = mybir.dt.np(BF16)
NPF8 = mybir.dt.np(F8)


def build_program(ablate: str = "") -> bass.Bass:
    """ablate: comma-set of {muls, mm, stage2} to drop (sim ablations only)."""
    abl = set(ablate.split(",")) if ablate else set()
    nc = bacc.Bacc("TRN2", target_bir_lowering=False, debug=False)

    # big inputs, orientation B: [p = parity*64+n, j, ba]
    lg_d = nc.dram_tensor("lg", [128, TH, BA], BF16, kind="ExternalInput")
    qv_d = nc.dram_tensor("qv", [128, TH, BA], BF16, kind="ExternalInput")
    tq_d = nc.dram_tensor("tq", [128, TH, BA], F8, kind="ExternalInput")
    oh_d = nc.dram_tensor("oh", [128, TH, BA], F8, kind="ExternalInput")
    # small inputs, orientation A: [ba, t]
    wgt = nc.dram_tensor("wgt", [BA, T], F32, kind="ExternalInput")
    rwd = nc.dram_tensor("rwd", [BA, T], F32, kind="ExternalInput")
    ones2_d = nc.dram_tensor("ones2", [128, 2], BF16, kind="ExternalInput")
    out = nc.dram_tensor("out", [BA, 3], F32, kind="ExternalOutput")

    AX = mybir.AxisListType.X
    OP = mybir.AluOpType

    with TileContext(nc) as tc:
        with (
            tc.tile_pool(name="inp", bufs=2) as inp,
            tc.tile_pool(name="prd", bufs=2) as prd,
            tc.tile_pool(name="per", bufs=1) as per,
            tc.tile_pool(name="ps", bufs=4, space="PSUM") as pspool,
        ):
            # parity selector: ones2[c, par] = 1 iff c//64 == par
            ones2 = per.tile([128, 2], BF16)
            nc.sync.dma_start(out=ones2[:], in_=ones2_d[:])

            w_t = per.tile([BA, T], F32)
            nc.sync.dma_start(out=w_t[:], in_=wgt[:])
            r_t = per.tile([BA, T], F32)
            nc.sync.dma_start(out=r_t[:], in_=rwd[:])

            # fields[ba, j, 2q+par] = quantity q at t=2j+par, f32
            # q: 0=sum_e 1=dot_eq 2=dot_el 3=q_tk 4=tq_tk 5=l_tk
            # split in T-halves so stage 2 on half A overlaps half-B streaming
            TH2 = TH // 2
            fieldsA = per.tile([BA, TH2, 12], F32)
            fieldsB = per.tile([BA, TH2, 12], F32)

            for c in range(NCH):
                sl = slice(c * TCH, (c + 1) * TCH)
                half, hsl = (
                    (fieldsA, slice(c * TCH, (c + 1) * TCH))
                    if c < NCH // 2
                    else (fieldsB, slice(c * TCH - TH2, (c + 1) * TCH - TH2))
                )

                lg = inp.tile([128, TCH, BA], BF16, tag="lg")
                qv = inp.tile([128, TCH, BA], BF16, tag="qv")
                tq = inp.tile([128, TCH, BA], F8, tag="tq")
                oh = inp.tile([128, TCH, BA], F8, tag="oh")
                nc.sync.dma_start(out=lg[:], in_=lg_d[:, sl])
                nc.sync.dma_start(out=qv[:], in_=qv_d[:, sl])
                nc.sync.dma_start(out=tq[:], in_=tq_d[:, sl])
                nc.sync.dma_start(out=oh[:], in_=oh_d[:, sl])

                # P[:, q] = the six streams: e, e*q, e*lg, oh*q, oh*tq, oh*lg
                P = prd.tile([128, 6, TCH, BA], BF16, tag="P")
                nc.scalar.activation(
                    out=P[:, 0], in_=lg[:], func=mybir.ActivationFunctionType.Exp
                )
                if "muls" not in abl:
                    # DVE (bf16 2x): e*lg, e*q
                    nc.vector.tensor_mul(P[:, 2], P[:, 0], lg[:])
                    nc.vector.tensor_mul(P[:, 1], P[:, 0], qv[:])
                    # Pool: oh*tq, oh*lg; oh*q alternates (DVE 1x vs Pool)
                    if c % 4 == 3:
                        nc.gpsimd.tensor_mul(P[:, 3], oh[:], qv[:])
                    else:
                        nc.vector.tensor_mul(P[:, 3], oh[:], qv[:])
                    nc.gpsimd.tensor_mul(P[:, 4], oh[:], tq[:])
                    nc.gpsimd.tensor_mul(P[:, 5], oh[:], lg[:])

                if "mm" not in abl:
                    psum = pspool.tile([128, TCH, 12], F32, tag="acc")
                    for j in range(TCH):
                        for q in range(6):
                            nc.tensor.matmul(
                                psum[:, j, 2 * q : 2 * q + 2],
                                P[:, q, j, :],
                                ones2[:],
                                start=True,
                                stop=True,
                            )
                    nc.scalar.copy(out=half[:, hsl], in_=psum[:])
                else:
                    nc.scalar.copy(
                        out=half[:, hsl],
                        in_=P[:, 0:1, :, 0:12].rearrange("p q a b -> p (q a) b"),
                    )
            if "stage2" in abl:
                partials = per.tile([BA, 3], F32)
                nc.vector.tensor_copy(partials[:], fieldsB[:, 0, 0:3])
                nc.sync.dma_start(out=out[:], in_=partials[:])
                return nc

            # ---- stage 2: per-(t,ba) scalar math ------------------------
            # fields views are logically t-ordered: [BA, j, par] == [BA, t].
            # Everything except the +-1-shifted lambda-return inputs reads the
            # fields tiles directly; processed per T-half so half A overlaps
            # half-B streaming.
            z = per.tile([BA, T], F32)
            rs = per.tile([BA, T], F32)
            logp = per.tile([BA, T], F32)
            adv = per.tile([BA, T], F32)
            ent = per.tile([BA, T], F32)
            # pol/qd/entw as slabs of one tile so one reduce covers all three
            acc3 = per.tile([BA, 3, T], F32)
            pol = acc3[:, 0]
            qd = acc3[:, 1]
            entw = acc3[:, 2]
            nc.vector.memset(acc3[:, 1, T - 1 : T], 0.0)
            q_tk = per.tile([BA, T], F32)
            tq_tk = per.tile([BA, T], F32)

            def half2(ap, h):
                return ap[:, h * T // 2 : (h + 1) * T // 2].rearrange(
                    "p (a b) -> p a b", a=TH2, b=2
                )

            for h, F in ((0, fieldsA), (1, fieldsB)):
                def fv(q):
                    return F[:, :, 2 * q : 2 * q + 2]

                zv, rsv = half2(z, h), half2(rs, h)
                logpv, advv = half2(logp, h), half2(adv, h)
                entv, polv, entwv = half2(ent, h), half2(pol, h), half2(entw, h)
                wv = half2(w_t, h)
                nc.scalar.activation(
                    out=zv, in_=fv(0), func=mybir.ActivationFunctionType.Ln
                )
                nc.vector.reciprocal(rsv, fv(0))
                nc.gpsimd.tensor_copy(half2(q_tk, h), fv(3))
                nc.gpsimd.tensor_copy(half2(tq_tk, h), fv(4))
                nc.vector.tensor_tensor(out=logpv, in0=fv(5), in1=zv, op=OP.subtract)
                nc.vector.tensor_mul(advv, fv(1), rsv)  # baseline
                nc.vector.tensor_tensor(out=advv, in0=fv(3), in1=advv, op=OP.subtract)
                nc.vector.tensor_mul(entv, fv(2), rsv)
                nc.vector.tensor_tensor(out=entv, in0=zv, in1=entv, op=OP.subtract)
                nc.vector.tensor_mul(polv, logpv, advv)
                nc.vector.tensor_mul(polv, polv, wv)
                nc.gpsimd.tensor_mul(entwv, entv, wv)

            # lambda returns (reverse-time scan via negative-step views)
            d = per.tile([BA, T - 1], F32)
            nc.gpsimd.tensor_scalar_mul(d[:], tq_tk[:, 1:T], GAMMA * (1.0 - LAMBDA))
            nc.gpsimd.tensor_add(d[:], d[:], r_t[:, 0 : T - 1])
            gl = per.tile([BA, 1], F32)
            nc.vector.memset(gl[:], GAMMA * LAMBDA)
            ret = per.tile([BA, T - 1], F32)
            nc.vector.tensor_tensor_scan(
                out=ret[:, ::-1],
                data0=gl[:].to_broadcast([BA, T - 1]),
                data1=d[:, ::-1],
                initial=tq_tk[:, T - 1 : T],
                op0=OP.mult,
                op1=OP.add,
            )

            qdv = qd[:, 0 : T - 1]
            nc.vector.tensor_tensor(
                out=qdv, in0=ret[:], in1=q_tk[:, 0 : T - 1], op=OP.subtract
            )
            nc.scalar.square(qdv, qdv)
            nc.vector.tensor_mul(qdv, qdv, w_t[:, 0 : T - 1])

            partials = per.tile([BA, 3], F32)
            nc.vector.reduce_sum(out=partials[:], in_=acc3[:], axis=AX)
            nc.sync.dma_start(out=out[:], in_=partials[:])

    return nc


def _orientB(x, dtype=None):
    """[T, BA, N] -> [parity*64+n, T//2, BA] contiguous."""
    y = x.reshape(T // 2, 2, BA, N).transpose(1, 3, 0, 2).reshape(128, TH, BA)
    return np.ascontiguousarray(y.astype(dtype if dtype is not None else NPBF16))


def make_in_maps(logit, action, q_value, target_q_value, reward, weight):
    logit = np.asarray(logit, np.float32)
    q_value = np.asarray(q_value, np.float32)
    target_q_value = np.asarray(target_q_value, np.float32)
    action = np.asarray(action)
    reward = np.asarray(reward, np.float32)
    weight = np.asarray(weight, np.float32)

    onehot = (action[..., None] == np.arange(N)).astype(np.float32)  # [T,B,A,N]

    ones2 = np.zeros((128, 2), np.float32)
    ones2[:64, 0] = 1.0
    ones2[64:, 1] = 1.0
    ones2 = ones2.astype(NPBF16)

    in_maps = []
    for r in range(M):
        bs, be = r * BL, (r + 1) * BL
        in_maps.append(
            {
                "ones2": ones2,
                "lg": _orientB(logit[:, bs:be].reshape(T, BA, N)),
                "qv": _orientB(q_value[:, bs:be].reshape(T, BA, N)),
                "tq": _orientB(target_q_value[:, bs:be].reshape(T, BA, N), NPF8),
                "oh": _orientB(onehot[:, bs:be].reshape(T, BA, N), NPF8),
                "wgt": np.ascontiguousarray(weight[:, bs:be].reshape(T, BA).T),
                "rwd": np.ascontiguousarray(
                    np.repeat(reward[:, bs:be], A, axis=1).T
                ),
            }
        )
    return in_maps


def combine_partials(partials_per_core):
    s = np.stack(partials_per_core).astype(np.float64).sum(axis=(0, 1))
    policy_loss = np.float32(-s[0] / (T * B * A))
    q_value_loss = np.float32(s[1] / ((T - 1) * B * A))
    entropy_loss = np.float32(s[2] / (T * B * A))
    return policy_loss, q_value_loss, entropy_loss


_program_cache = {}


def _get_program() -> bass.Bass:
    if "nc" not in _program_cache:
        nc = build_program()
        nc.finalize()
        _program_cache["nc"] = nc
    return _program_cache["nc"]


def kernel(logit, action, q_value, target_q_value, reward, weight):
    nc = _get_program()
    in_maps = make_in_maps(logit, action, q_value, target_q_value, reward, weight)
    res = run_bass_kernel_spmd(nc, in_maps, list(range(M))).results
    return combine_partials([np.asarray(res[i]["out"]) for i in range(M)])
